# revision 6
# baseline (speedup 1.0000x reference)
"""HarmonicEvolutionLayer on 8 trn2 NeuronCores.

Math: out = LN(einsum(Re(ifft(fft(x_quat, seq) * K, seq)), R)).
The FFT->K->IFFT chain is a circular convolution along seq with the real
taps h = Re(ifft(K)).  For the actual inputs (K = ones) h is a delta, and
R = eye, gamma = 1, beta = 0 -- so the device kernel only needs a
row-wise LayerNorm.  All of that structure is *detected at runtime* from
the input values; non-trivial taps / rotation / affine fall back to a
general path so the kernel stays correct for arbitrary parameter values.

Sharding: rows of the flattened (B*S, D) = (16384, 1024) tensor are split
8 ways (data-parallel; LN is per-row), 2048 rows per core.

I/O precision: fp16 both ways (host casts); stats in fp32 on-chip.

Work split (HBM pace is ~23.5us for 8.4MB; keep every engine below it):
  vector  bn_stats/bn_aggr for 13 of 16 row-groups (+ tiny stats math)
  scalar  Copy+accum / Square+accum stats for 3 row-groups, 11 of 16
          normalization applies (Identity activation), Sqrt
  gpsimd  5 of 16 applies (tensor_scalar on Q7)
  sync    all input loads (issued first, no waits) and all stores
"""

import sys

import numpy as np

for _p in ("/opt/trn_rl_repo",):
    if _p not in sys.path:
        sys.path.insert(0, _p)

import concourse.bass as bass
from concourse import bacc, mybir
from concourse.tile import TileContext
from concourse.bass_utils import run_bass_kernel_spmd

B, S, D = 4, 4096, 1024
ROT = 4
EPS = 1e-5
N_CORES = 8
ROWS_PER_CORE = (B * S) // N_CORES      # 2048
P = 128                                 # SBUF partitions
TILE_J = 4                              # rows per partition per tile
N_TILES = ROWS_PER_CORE // (P * TILE_J)  # 4

# (tile, j) assignment: stats on scalar for these groups
SCALAR_STATS = {(0, 3), (1, 3), (2, 3)}
# apply engine per (tile, j): 'S' scalar, 'G' gpsimd (rest scalar)
GP_APPLY = {(0, 2), (0, 3), (1, 3), (2, 3), (3, 2)}

_nc_cache: dict = {}


def _build_nc(scale: float, affine: bool) -> bass.Bass:
    """Per-core program: rows (2048, 1024) fp16 -> LayerNorm -> fp16."""
    nc = bacc.Bacc("TRN2", target_bir_lowering=False, debug=False,
                   num_devices=N_CORES)
    x = nc.dram_tensor("x", [ROWS_PER_CORE, D], mybir.dt.float16,
                       kind="ExternalInput")
    out = nc.dram_tensor("out", [ROWS_PER_CORE, D], mybir.dt.float16,
                         kind="ExternalOutput")
    if affine:
        gamma = nc.dram_tensor("gamma", [P, D], mybir.dt.float32,
                               kind="ExternalInput")
        beta = nc.dram_tensor("beta", [P, D], mybir.dt.float32,
                              kind="ExternalInput")

    x_r = x.rearrange("(n p j) d -> n p j d", p=P, j=TILE_J)
    out_r = out.rearrange("(n p j) d -> n p j d", p=P, j=TILE_J)

    FMAX = nc.vector.BN_STATS_FMAX          # 512
    n_sub = D // FMAX                       # 2
    SDIM = nc.vector.BN_STATS_DIM           # 6
    ADIM = nc.vector.BN_AGGR_DIM            # 2

    with TileContext(nc) as tc:
        with (
            tc.tile_pool(name="work", bufs=1) as work,
            tc.tile_pool(name="small", bufs=4) as small,
            tc.tile_pool(name="singles", bufs=1) as singles,
        ):
            # input loads first: no dependencies, sync engine idles
            xts = []
            for i in range(N_TILES):
                xt = work.tile([P, TILE_J, D], mybir.dt.float16,
                               tag=f"xt{i}")
                nc.sync.dma_start(out=xt, in_=x_r[i])
                xts.append(xt)

            eps_t = singles.tile([P, 1], mybir.dt.float32)
            nc.vector.memset(eps_t, EPS)
            sq_sink = singles.tile([P, D], mybir.dt.float16)
            if affine:
                gamma_t = singles.tile([P, D], mybir.dt.float32)
                beta_t = singles.tile([P, D], mybir.dt.float32)
                nc.sync.dma_start(out=gamma_t, in_=gamma[:, :])
                nc.sync.dma_start(out=beta_t, in_=beta[:, :])

            for i in range(N_TILES):
                xt = xts[i]
                if scale != 1.0:
                    nc.scalar.mul(out=xt, in_=xt, mul=scale)
                # mv[:, j, 0] = mean, mv[:, j, 1] = var for every j
                stats = small.tile([P, TILE_J, n_sub, SDIM],
                                   mybir.dt.float32)
                mv = small.tile([P, TILE_J, ADIM], mybir.dt.float32)
                sums = small.tile([P, TILE_J], mybir.dt.float32)
                ssq = small.tile([P, TILE_J], mybir.dt.float32)
                m2 = small.tile([P, TILE_J], mybir.dt.float32)
                for j in range(TILE_J):
                    if (i, j) in SCALAR_STATS:
                        nc.scalar.activation(
                            out=sq_sink, in_=xt[:, j, :],
                            func=mybir.ActivationFunctionType.Copy,
                            accum_out=sums[:, j:j + 1],
                        )
                        nc.scalar.activation(
                            out=sq_sink, in_=xt[:, j, :],
                            func=mybir.ActivationFunctionType.Square,
                            accum_out=ssq[:, j:j + 1],
                        )
                        # mean, var from the two accumulators (vector)
                        nc.vector.tensor_scalar_mul(
                            out=mv[:, j, 0:1], in0=sums[:, j:j + 1],
                            scalar1=1.0 / D)
                        nc.vector.tensor_tensor(
                            out=m2[:, j:j + 1], in0=mv[:, j, 0:1],
                            in1=mv[:, j, 0:1], op=mybir.AluOpType.mult)
                        nc.vector.tensor_scalar_mul(
                            out=mv[:, j, 1:2], in0=ssq[:, j:j + 1],
                            scalar1=1.0 / D)
                        nc.vector.tensor_tensor(
                            out=mv[:, j, 1:2], in0=mv[:, j, 1:2],
                            in1=m2[:, j:j + 1],
                            op=mybir.AluOpType.subtract)
                    else:
                        for k in range(n_sub):
                            nc.vector.bn_stats(
                                out=stats[:, j, k, :],
                                in_=xt[:, j, k * FMAX:(k + 1) * FMAX],
                            )
                        nc.vector.bn_aggr(out=mv[:, j, :],
                                          in_=stats[:, j, :, :])
                # rstd = 1/sqrt(var + eps); nmb = -mean*rstd (scalar bias)
                std = small.tile([P, TILE_J], mybir.dt.float32)
                rstd = small.tile([P, TILE_J], mybir.dt.float32)
                nmean = small.tile([P, TILE_J], mybir.dt.float32)
                nmb = small.tile([P, TILE_J], mybir.dt.float32)
                nc.scalar.activation(
                    out=std, in_=mv[:, :, 1],
                    func=mybir.ActivationFunctionType.Sqrt,
                    bias=eps_t[:, 0:1], scale=1.0,
                )
                nc.vector.reciprocal(out=rstd, in_=std)
                nc.vector.tensor_scalar_mul(out=nmean, in0=mv[:, :, 0],
                                            scalar1=-1.0)
                nc.vector.tensor_tensor(out=nmb, in0=nmean, in1=rstd,
                                        op=mybir.AluOpType.mult)
                yt = work.tile([P, TILE_J, D], mybir.dt.float16,
                               tag=f"yt{i}")
                for j in range(TILE_J):
                    if (i, j) in GP_APPLY:
                        nc.gpsimd.tensor_scalar(
                            out=yt[:, j, :], in0=xt[:, j, :],
                            scalar1=mv[:, j, 0:1],
                            scalar2=rstd[:, j:j + 1],
                            op0=mybir.AluOpType.subtract,
                            op1=mybir.AluOpType.mult,
                        )
                    else:
                        nc.scalar.activation(
                            out=yt[:, j, :], in_=xt[:, j, :],
                            func=mybir.ActivationFunctionType.Identity,
                            bias=nmb[:, j:j + 1],
                            scale=rstd[:, j:j + 1],
                        )
                    if affine:
                        nc.vector.tensor_mul(out=yt[:, j, :],
                                             in0=yt[:, j, :], in1=gamma_t)
                        nc.vector.tensor_add(out=yt[:, j, :],
                                             in0=yt[:, j, :], in1=beta_t)
                    if j % 2 == 1:
                        nc.sync.dma_start(
                            out=out_r[i, :, j - 1:j + 1, :],
                            in_=yt[:, j - 1:j + 1, :])
    nc.compile()
    return nc


def _get_nc(scale: float, affine: bool) -> bass.Bass:
    key = (round(scale, 12), affine)
    if key not in _nc_cache:
        _nc_cache[key] = _build_nc(scale, affine)
    return _nc_cache[key]


def _preprocess(x, rotation_matrix, frequency_kernel):
    """Fold the frequency filter + rotation into (y, scale) on the host."""
    b, s, d = x.shape
    K = np.asarray(frequency_kernel, np.float64)[:s]
    h = np.fft.ifft(K).real
    y = x
    scale = float(h[0])
    if np.max(np.abs(h[1:])) > 1e-9 * max(1.0, np.max(np.abs(h))):
        xq = x.reshape(b, s, d // ROT, ROT)
        y = np.fft.ifft(np.fft.fft(xq, axis=1) * K.reshape(1, s, 1, 1),
                        axis=1).real.astype(np.float32).reshape(b, s, d)
        scale = 1.0
    R = np.asarray(rotation_matrix, np.float32)
    if not np.allclose(R, np.eye(ROT, dtype=np.float32), atol=1e-9):
        y = np.einsum("bstq,oq->bsto", y.reshape(b, s, d // ROT, ROT),
                      R).reshape(b, s, d).astype(np.float32)
    return np.ascontiguousarray(y, np.float32), scale


def run(x, rotation_matrix, frequency_kernel, ln_gamma, ln_beta,
        trace: bool = False, tmpdir: str | None = None):
    x = np.ascontiguousarray(np.asarray(x, np.float32))
    assert x.shape == (B, S, D), x.shape
    y, scale = _preprocess(x, rotation_matrix, frequency_kernel)
    if abs(scale - 1.0) < 1e-12:
        scale = 1.0
    g = np.asarray(ln_gamma, np.float32)
    bt = np.asarray(ln_beta, np.float32)
    affine = not (np.all(g == 1.0) and np.all(bt == 0.0))

    nc = _get_nc(scale, affine)
    y16 = y.astype(np.float16)
    shards = y16.reshape(N_CORES, ROWS_PER_CORE, D)
    in_maps = []
    for c in range(N_CORES):
        m = {"x": shards[c]}
        if affine:
            m["gamma"] = np.ascontiguousarray(
                np.broadcast_to(g, (P, D)), np.float32)
            m["beta"] = np.ascontiguousarray(
                np.broadcast_to(bt, (P, D)), np.float32)
        in_maps.append(m)
    res = run_bass_kernel_spmd(nc, in_maps, list(range(N_CORES)),
                               trace=trace, tmpdir=tmpdir)
    out = np.stack([res.results[c]["out"] for c in range(N_CORES)])
    return out.reshape(B, S, D).astype(np.float32), res


def kernel(x, rotation_matrix, frequency_kernel, ln_gamma, ln_beta):
    out, _ = run(x, rotation_matrix, frequency_kernel, ln_gamma, ln_beta)
    return out


# revision 7
# speedup vs baseline: 2.1778x; 2.1778x over previous
"""HarmonicEvolutionLayer on 8 trn2 NeuronCores.

Math: out = LN(einsum(Re(ifft(fft(x_quat, seq) * K, seq)), R)).
The FFT->K->IFFT chain is a circular convolution along seq with the real
taps h = Re(ifft(K)).  For the actual inputs (K = ones) h is a delta, and
R = eye, gamma = 1, beta = 0 -- so the device kernel only needs a
row-wise LayerNorm.  All of that structure is *detected at runtime* from
the input values; non-trivial taps / rotation / affine fall back to a
general path so the kernel stays correct for arbitrary parameter values.

Sharding: rows of the flattened (B*S, D) = (16384, 1024) tensor are split
8 ways (data-parallel; LN is per-row), 2048 rows per core.

I/O precision: fp16 both ways (host casts); stats in fp32 on-chip.

Work split (HBM pace is ~23.5us for 8.4MB; keep every engine below it):
  vector  bn_stats/bn_aggr for 13 of 16 row-groups (+ tiny stats math)
  scalar  Copy+accum / Square+accum stats for 3 row-groups, 11 of 16
          normalization applies (Identity activation), Sqrt
  gpsimd  5 of 16 applies (tensor_scalar on Q7)
  sync    all input loads (issued first, no waits) and all stores
"""

import sys

import numpy as np

for _p in ("/opt/trn_rl_repo",):
    if _p not in sys.path:
        sys.path.insert(0, _p)

import concourse.bass as bass
from concourse import bacc, mybir
from concourse.tile import TileContext
from concourse.bass_utils import run_bass_kernel_spmd

B, S, D = 4, 4096, 1024
ROT = 4
EPS = 1e-5
N_CORES = 8
ROWS_PER_CORE = (B * S) // N_CORES      # 2048
P = 128                                 # SBUF partitions
TILE_J = 4                              # rows per partition per tile
N_TILES = ROWS_PER_CORE // (P * TILE_J)  # 4

# (tile, j) assignment: stats on scalar for these groups (gpsimd's Q7
# tensor ops measured ~15us each -- unusable; scalar-stats kept empty to
# avoid Square/Sqrt ACT-table thrash)
SCALAR_STATS: set = set()
GP_APPLY: set = set()

_nc_cache: dict = {}


def _build_nc(scale: float, affine: bool) -> bass.Bass:
    """Per-core program: rows (2048, 1024) fp16 -> LayerNorm -> fp16."""
    nc = bacc.Bacc("TRN2", target_bir_lowering=False, debug=False,
                   num_devices=N_CORES)
    x = nc.dram_tensor("x", [ROWS_PER_CORE, D], mybir.dt.float16,
                       kind="ExternalInput")
    out = nc.dram_tensor("out", [ROWS_PER_CORE, D], mybir.dt.float16,
                         kind="ExternalOutput")
    if affine:
        gamma = nc.dram_tensor("gamma", [P, D], mybir.dt.float32,
                               kind="ExternalInput")
        beta = nc.dram_tensor("beta", [P, D], mybir.dt.float32,
                              kind="ExternalInput")

    x_r = x.rearrange("(n p j) d -> n p j d", p=P, j=TILE_J)
    out_r = out.rearrange("(n p j) d -> n p j d", p=P, j=TILE_J)

    FMAX = nc.vector.BN_STATS_FMAX          # 512
    n_sub = D // FMAX                       # 2
    SDIM = nc.vector.BN_STATS_DIM           # 6
    ADIM = nc.vector.BN_AGGR_DIM            # 2

    with TileContext(nc) as tc:
        with (
            tc.tile_pool(name="work", bufs=1) as work,
            tc.tile_pool(name="small", bufs=4) as small,
            tc.tile_pool(name="singles", bufs=1) as singles,
        ):
            # input loads first: no dependencies, sync engine idles
            xts = []
            for i in range(N_TILES):
                xt = work.tile([P, TILE_J, D], mybir.dt.float16,
                               tag=f"xt{i}")
                nc.sync.dma_start(out=xt, in_=x_r[i])
                xts.append(xt)

            eps_t = singles.tile([P, 1], mybir.dt.float32)
            nc.vector.memset(eps_t, EPS)
            sq_sink = singles.tile([P, D], mybir.dt.float16)
            if affine:
                gamma_t = singles.tile([P, D], mybir.dt.float32)
                beta_t = singles.tile([P, D], mybir.dt.float32)
                nc.sync.dma_start(out=gamma_t, in_=gamma[:, :])
                nc.sync.dma_start(out=beta_t, in_=beta[:, :])

            for i in range(N_TILES):
                xt = xts[i]
                if scale != 1.0:
                    nc.scalar.mul(out=xt, in_=xt, mul=scale)
                # mv[:, j, 0] = mean, mv[:, j, 1] = var for every j
                stats = small.tile([P, TILE_J, n_sub, SDIM],
                                   mybir.dt.float32)
                mv = small.tile([P, TILE_J, ADIM], mybir.dt.float32)
                sums = small.tile([P, TILE_J], mybir.dt.float32)
                ssq = small.tile([P, TILE_J], mybir.dt.float32)
                m2 = small.tile([P, TILE_J], mybir.dt.float32)
                for j in range(TILE_J):
                    if (i, j) in SCALAR_STATS:
                        nc.scalar.activation(
                            out=sq_sink, in_=xt[:, j, :],
                            func=mybir.ActivationFunctionType.Copy,
                            accum_out=sums[:, j:j + 1],
                        )
                        nc.scalar.activation(
                            out=sq_sink, in_=xt[:, j, :],
                            func=mybir.ActivationFunctionType.Square,
                            accum_out=ssq[:, j:j + 1],
                        )
                        # mean, var from the two accumulators (vector)
                        nc.vector.tensor_scalar_mul(
                            out=mv[:, j, 0:1], in0=sums[:, j:j + 1],
                            scalar1=1.0 / D)
                        nc.vector.tensor_tensor(
                            out=m2[:, j:j + 1], in0=mv[:, j, 0:1],
                            in1=mv[:, j, 0:1], op=mybir.AluOpType.mult)
                        nc.vector.tensor_scalar_mul(
                            out=mv[:, j, 1:2], in0=ssq[:, j:j + 1],
                            scalar1=1.0 / D)
                        nc.vector.tensor_tensor(
                            out=mv[:, j, 1:2], in0=mv[:, j, 1:2],
                            in1=m2[:, j:j + 1],
                            op=mybir.AluOpType.subtract)
                    else:
                        for k in range(n_sub):
                            nc.vector.bn_stats(
                                out=stats[:, j, k, :],
                                in_=xt[:, j, k * FMAX:(k + 1) * FMAX],
                            )
                        nc.vector.bn_aggr(out=mv[:, j, :],
                                          in_=stats[:, j, :, :])
                # rstd = 1/sqrt(var + eps); nmb = -mean*rstd (scalar bias)
                std = small.tile([P, TILE_J], mybir.dt.float32)
                rstd = small.tile([P, TILE_J], mybir.dt.float32)
                nmean = small.tile([P, TILE_J], mybir.dt.float32)
                nmb = small.tile([P, TILE_J], mybir.dt.float32)
                nc.scalar.activation(
                    out=std, in_=mv[:, :, 1],
                    func=mybir.ActivationFunctionType.Sqrt,
                    bias=eps_t[:, 0:1], scale=1.0,
                )
                nc.vector.reciprocal(out=rstd, in_=std)
                nc.vector.tensor_scalar_mul(out=nmean, in0=mv[:, :, 0],
                                            scalar1=-1.0)
                nc.vector.tensor_tensor(out=nmb, in0=nmean, in1=rstd,
                                        op=mybir.AluOpType.mult)
                yt = work.tile([P, TILE_J, D], mybir.dt.float16,
                               tag=f"yt{i}")
                for j in range(TILE_J):
                    if (i, j) in GP_APPLY:
                        nc.gpsimd.tensor_scalar(
                            out=yt[:, j, :], in0=xt[:, j, :],
                            scalar1=mv[:, j, 0:1],
                            scalar2=rstd[:, j:j + 1],
                            op0=mybir.AluOpType.subtract,
                            op1=mybir.AluOpType.mult,
                        )
                    else:
                        nc.scalar.activation(
                            out=yt[:, j, :], in_=xt[:, j, :],
                            func=mybir.ActivationFunctionType.Identity,
                            bias=nmb[:, j:j + 1],
                            scale=rstd[:, j:j + 1],
                        )
                    if affine:
                        nc.vector.tensor_mul(out=yt[:, j, :],
                                             in0=yt[:, j, :], in1=gamma_t)
                        nc.vector.tensor_add(out=yt[:, j, :],
                                             in0=yt[:, j, :], in1=beta_t)
                    if j % 2 == 1:
                        nc.sync.dma_start(
                            out=out_r[i, :, j - 1:j + 1, :],
                            in_=yt[:, j - 1:j + 1, :])
    nc.compile()
    return nc


def _get_nc(scale: float, affine: bool) -> bass.Bass:
    key = (round(scale, 12), affine)
    if key not in _nc_cache:
        _nc_cache[key] = _build_nc(scale, affine)
    return _nc_cache[key]


def _preprocess(x, rotation_matrix, frequency_kernel):
    """Fold the frequency filter + rotation into (y, scale) on the host."""
    b, s, d = x.shape
    K = np.asarray(frequency_kernel, np.float64)[:s]
    h = np.fft.ifft(K).real
    y = x
    scale = float(h[0])
    if np.max(np.abs(h[1:])) > 1e-9 * max(1.0, np.max(np.abs(h))):
        xq = x.reshape(b, s, d // ROT, ROT)
        y = np.fft.ifft(np.fft.fft(xq, axis=1) * K.reshape(1, s, 1, 1),
                        axis=1).real.astype(np.float32).reshape(b, s, d)
        scale = 1.0
    R = np.asarray(rotation_matrix, np.float32)
    if not np.allclose(R, np.eye(ROT, dtype=np.float32), atol=1e-9):
        y = np.einsum("bstq,oq->bsto", y.reshape(b, s, d // ROT, ROT),
                      R).reshape(b, s, d).astype(np.float32)
    return np.ascontiguousarray(y, np.float32), scale


def run(x, rotation_matrix, frequency_kernel, ln_gamma, ln_beta,
        trace: bool = False, tmpdir: str | None = None):
    x = np.ascontiguousarray(np.asarray(x, np.float32))
    assert x.shape == (B, S, D), x.shape
    y, scale = _preprocess(x, rotation_matrix, frequency_kernel)
    if abs(scale - 1.0) < 1e-12:
        scale = 1.0
    g = np.asarray(ln_gamma, np.float32)
    bt = np.asarray(ln_beta, np.float32)
    affine = not (np.all(g == 1.0) and np.all(bt == 0.0))

    nc = _get_nc(scale, affine)
    y16 = y.astype(np.float16)
    shards = y16.reshape(N_CORES, ROWS_PER_CORE, D)
    in_maps = []
    for c in range(N_CORES):
        m = {"x": shards[c]}
        if affine:
            m["gamma"] = np.ascontiguousarray(
                np.broadcast_to(g, (P, D)), np.float32)
            m["beta"] = np.ascontiguousarray(
                np.broadcast_to(bt, (P, D)), np.float32)
        in_maps.append(m)
    res = run_bass_kernel_spmd(nc, in_maps, list(range(N_CORES)),
                               trace=trace, tmpdir=tmpdir)
    out = np.stack([res.results[c]["out"] for c in range(N_CORES)])
    return out.reshape(B, S, D).astype(np.float32), res


def kernel(x, rotation_matrix, frequency_kernel, ln_gamma, ln_beta):
    out, _ = run(x, rotation_matrix, frequency_kernel, ln_gamma, ln_beta)
    return out


# revision 11
# speedup vs baseline: 2.3626x; 1.0849x over previous
"""HarmonicEvolutionLayer on 8 trn2 NeuronCores.

Math: out = LN(einsum(Re(ifft(fft(x_quat, seq) * K, seq)), R)).
The FFT->K->IFFT chain is a circular convolution along seq with the real
taps h = Re(ifft(K)).  For the actual inputs (K = ones) h is a delta, and
R = eye, gamma = 1, beta = 0 -- so the device kernel only needs a
row-wise LayerNorm.  All of that structure is *detected at runtime* from
the input values; non-trivial taps / rotation / affine fall back to a
general path so the kernel stays correct for arbitrary parameter values.

Sharding: rows of the flattened (B*S, D) = (16384, 1024) tensor are split
8 ways (data-parallel; LN is per-row), 2048 rows per core.

I/O precision: fp16 both ways (host casts); stats in fp32 on-chip.

Pipeline (HBM pace ~23.5us for 8.4MB, both compute engines just below):
  vector  bn_stats/bn_aggr for all row-groups + reciprocal + tiny math
  scalar  all 16 normalization applies (Identity activation) + Sqrt
  sync    all loads (emitted first, FIFO => tile0 arrives earliest),
          then stores
Tiles are laddered [1,3,4,4,3,1] j-groups (j-group = 128 rows): a small
first tile starts compute ~3us earlier, a small last tile shortens the
stats->sqrt->apply->store drain; emission is software-pipelined so each
engine's in-order queue never parks on a far-future dependency.
"""

import sys

import numpy as np

for _p in ("/opt/trn_rl_repo",):
    if _p not in sys.path:
        sys.path.insert(0, _p)

import concourse.bass as bass
from concourse import bacc, mybir
from concourse.tile import TileContext
from concourse.bass_utils import run_bass_kernel_spmd

B, S, D = 4, 4096, 1024
ROT = 4
EPS = 1e-5
N_CORES = 8
ROWS_PER_CORE = (B * S) // N_CORES      # 2048
P = 128                                 # SBUF partitions
TILE_JS = [1, 3, 4, 4, 3, 1]            # j-groups per tile (sum 16)
N_J = ROWS_PER_CORE // P                # 16

_nc_cache: dict = {}


def _build_nc(scale: float, affine: bool) -> bass.Bass:
    """Per-core program: rows (2048, 1024) fp16 -> LayerNorm -> fp16."""
    nc = bacc.Bacc("TRN2", target_bir_lowering=False, debug=False,
                   num_devices=N_CORES)
    x = nc.dram_tensor("x", [ROWS_PER_CORE, D], mybir.dt.float16,
                       kind="ExternalInput")
    out = nc.dram_tensor("out", [ROWS_PER_CORE, D], mybir.dt.float16,
                         kind="ExternalOutput")
    if affine:
        gamma = nc.dram_tensor("gamma", [P, D], mybir.dt.float32,
                               kind="ExternalInput")
        beta = nc.dram_tensor("beta", [P, D], mybir.dt.float32,
                              kind="ExternalInput")

    # tile i covers rows [128*off_i, 128*(off_i+njs)); within a tile,
    # partition p holds njs consecutive DRAM rows (contiguous DMA lines)

    FMAX = nc.vector.BN_STATS_FMAX          # 512
    n_sub = D // FMAX                       # 2
    SDIM = nc.vector.BN_STATS_DIM           # 6
    ADIM = nc.vector.BN_AGGR_DIM            # 2

    offs = [0]
    for njs in TILE_JS:
        offs.append(offs[-1] + njs)
    assert offs[-1] == N_J

    with TileContext(nc) as tc:
        with (
            tc.tile_pool(name="work", bufs=1) as work,
            tc.tile_pool(name="small", bufs=1) as small,
            tc.tile_pool(name="singles", bufs=1) as singles,
        ):
            # all input loads first (no deps; FIFO order = tile order)
            xts = []
            x_views = []
            out_views = []
            for i, njs in enumerate(TILE_JS):
                xv = x[P * offs[i]:P * offs[i + 1], :].rearrange(
                    "(p j) d -> p j d", j=njs)
                ov = out[P * offs[i]:P * offs[i + 1], :].rearrange(
                    "(p j) d -> p j d", j=njs)
                x_views.append(xv)
                out_views.append(ov)
                xt = work.tile([P, njs, D], mybir.dt.float16,
                               tag=f"xt{i}")
                nc.sync.dma_start(out=xt, in_=xv)
                xts.append(xt)

            eps_t = singles.tile([P, 1], mybir.dt.float32)
            nc.vector.memset(eps_t, EPS)
            if affine:
                gamma_t = singles.tile([P, D], mybir.dt.float32)
                beta_t = singles.tile([P, D], mybir.dt.float32)
                nc.sync.dma_start(out=gamma_t, in_=gamma[:, :])
                nc.sync.dma_start(out=beta_t, in_=beta[:, :])

            # per-tile state for the software pipeline
            state = []
            for i, njs in enumerate(TILE_JS):
                mv = small.tile([P, njs, ADIM], mybir.dt.float32,
                                tag=f"mv{i}")
                std = small.tile([P, njs], mybir.dt.float32,
                                 tag=f"std{i}")
                rstd = small.tile([P, njs], mybir.dt.float32,
                                  tag=f"rstd{i}")
                nmean = small.tile([P, njs], mybir.dt.float32,
                                   tag=f"nmean{i}")
                nmb = small.tile([P, njs], mybir.dt.float32,
                                 tag=f"nmb{i}")
                yt = work.tile([P, njs, D], mybir.dt.float16,
                               tag=f"yt{i}")
                state.append((mv, std, rstd, nmean, nmb, yt))

            def emit_stats(i):
                njs = TILE_JS[i]
                xt = xts[i]
                mv, std, rstd, nmean, nmb, _ = state[i]
                if scale != 1.0:
                    nc.scalar.mul(out=xt, in_=xt, mul=scale)
                stats = small.tile([P, njs, n_sub, SDIM],
                                   mybir.dt.float32, tag=f"stats{i}")
                for j in range(njs):
                    for k in range(n_sub):
                        nc.vector.bn_stats(
                            out=stats[:, j, k, :],
                            in_=xt[:, j, k * FMAX:(k + 1) * FMAX],
                        )
                    nc.vector.bn_aggr(out=mv[:, j, :],
                                      in_=stats[:, j, :, :])
                # nmean right away (only needs mv)
                nc.vector.tensor_scalar_mul(out=nmean, in0=mv[:, :, 0],
                                            scalar1=-1.0)
                nc.scalar.activation(
                    out=std, in_=mv[:, :, 1],
                    func=mybir.ActivationFunctionType.Sqrt,
                    bias=eps_t[:, 0:1], scale=1.0,
                )
                nc.vector.reciprocal(out=rstd, in_=std)
                nc.vector.tensor_tensor(out=nmb, in0=nmean, in1=rstd,
                                        op=mybir.AluOpType.mult)

            def emit_apply(i):
                njs = TILE_JS[i]
                xt = xts[i]
                mv, std, rstd, nmean, nmb, yt = state[i]
                store_every = 1 if njs <= 2 else 2
                for j0 in range(0, njs, store_every):
                    j1 = min(j0 + store_every, njs)
                    for j in range(j0, j1):
                        nc.scalar.activation(
                            out=yt[:, j, :], in_=xt[:, j, :],
                            func=mybir.ActivationFunctionType.Identity,
                            bias=nmb[:, j:j + 1],
                            scale=rstd[:, j:j + 1],
                        )
                        if affine:
                            nc.vector.tensor_mul(out=yt[:, j, :],
                                                 in0=yt[:, j, :],
                                                 in1=gamma_t)
                            nc.vector.tensor_add(out=yt[:, j, :],
                                                 in0=yt[:, j, :],
                                                 in1=beta_t)
                    nc.sync.dma_start(out=out_views[i][:, j0:j1, :],
                                      in_=yt[:, j0:j1, :])

            # software pipeline: stats(i) ... then apply(i-1)
            emit_stats(0)
            for i in range(1, len(TILE_JS)):
                emit_stats(i)
                emit_apply(i - 1)
            emit_apply(len(TILE_JS) - 1)
    nc.compile()
    return nc


def _get_nc(scale: float, affine: bool) -> bass.Bass:
    key = (round(scale, 12), affine)
    if key not in _nc_cache:
        _nc_cache[key] = _build_nc(scale, affine)
    return _nc_cache[key]


def _preprocess(x, rotation_matrix, frequency_kernel):
    """Fold the frequency filter + rotation into (y, scale) on the host."""
    b, s, d = x.shape
    K = np.asarray(frequency_kernel, np.float64)[:s]
    h = np.fft.ifft(K).real
    y = x
    scale = float(h[0])
    if np.max(np.abs(h[1:])) > 1e-9 * max(1.0, np.max(np.abs(h))):
        xq = x.reshape(b, s, d // ROT, ROT)
        y = np.fft.ifft(np.fft.fft(xq, axis=1) * K.reshape(1, s, 1, 1),
                        axis=1).real.astype(np.float32).reshape(b, s, d)
        scale = 1.0
    R = np.asarray(rotation_matrix, np.float32)
    if not np.allclose(R, np.eye(ROT, dtype=np.float32), atol=1e-9):
        y = np.einsum("bstq,oq->bsto", y.reshape(b, s, d // ROT, ROT),
                      R).reshape(b, s, d).astype(np.float32)
    return np.ascontiguousarray(y, np.float32), scale


def run(x, rotation_matrix, frequency_kernel, ln_gamma, ln_beta,
        trace: bool = False, tmpdir: str | None = None):
    x = np.ascontiguousarray(np.asarray(x, np.float32))
    assert x.shape == (B, S, D), x.shape
    y, scale = _preprocess(x, rotation_matrix, frequency_kernel)
    if abs(scale - 1.0) < 1e-12:
        scale = 1.0
    g = np.asarray(ln_gamma, np.float32)
    bt = np.asarray(ln_beta, np.float32)
    affine = not (np.all(g == 1.0) and np.all(bt == 0.0))

    nc = _get_nc(scale, affine)
    y16 = y.astype(np.float16)
    # j-major shards: core c gets rows [c*2048, (c+1)*2048); within a
    # core, row r maps to (j = r // P, p = r % P)
    shards = y16.reshape(N_CORES, ROWS_PER_CORE, D)
    in_maps = []
    for c in range(N_CORES):
        m = {"x": shards[c]}
        if affine:
            m["gamma"] = np.ascontiguousarray(
                np.broadcast_to(g, (P, D)), np.float32)
            m["beta"] = np.ascontiguousarray(
                np.broadcast_to(bt, (P, D)), np.float32)
        in_maps.append(m)
    res = run_bass_kernel_spmd(nc, in_maps, list(range(N_CORES)),
                               trace=trace, tmpdir=tmpdir)
    out = np.stack([res.results[c]["out"] for c in range(N_CORES)])
    return out.reshape(B, S, D).astype(np.float32), res


def kernel(x, rotation_matrix, frequency_kernel, ln_gamma, ln_beta):
    out, _ = run(x, rotation_matrix, frequency_kernel, ln_gamma, ln_beta)
    return out


# revision 15
# speedup vs baseline: 2.5773x; 1.0909x over previous
"""HarmonicEvolutionLayer on 8 trn2 NeuronCores.

Math: out = LN(einsum(Re(ifft(fft(x_quat, seq) * K, seq)), R)).
The FFT->K->IFFT chain is a circular convolution along seq with the real
taps h = Re(ifft(K)).  For the actual inputs (K = ones) h is a delta, and
R = eye, gamma = 1, beta = 0 -- so the device kernel only needs a
row-wise LayerNorm.  All of that structure is *detected at runtime* from
the input values; non-trivial taps / rotation / affine fall back to a
general path so the kernel stays correct for arbitrary parameter values.

Sharding: rows of the flattened (B*S, D) = (16384, 1024) tensor are split
8 ways (data-parallel; LN is per-row), 2048 rows per core.

I/O precision: fp16 both ways (host casts); stats in fp32 on-chip.

Pipeline (HBM pace ~23.5us for 8.4MB, both compute engines just below):
  vector  bn_stats/bn_aggr for all row-groups + reciprocal + tiny math
  scalar  all 16 normalization applies (Identity activation) + Sqrt
  sync    all loads (emitted first, FIFO => tile0 arrives earliest),
          then stores
Tiles are laddered [1,3,4,4,3,1] j-groups (j-group = 128 rows): a small
first tile starts compute ~3us earlier, a small last tile shortens the
stats->sqrt->apply->store drain; emission is software-pipelined so each
engine's in-order queue never parks on a far-future dependency.
"""

import sys

import numpy as np

for _p in ("/opt/trn_rl_repo",):
    if _p not in sys.path:
        sys.path.insert(0, _p)

import concourse.bass as bass
from concourse import bacc, mybir
from concourse.tile import TileContext
from concourse.bass_utils import run_bass_kernel_spmd

B, S, D = 4, 4096, 1024
ROT = 4
EPS = 1e-5
N_CORES = 8
ROWS_PER_CORE = (B * S) // N_CORES      # 2048
P = 128                                 # SBUF partitions
TILE_JS = [1, 3, 4, 4, 3, 1]            # j-groups per tile (sum 16)
N_J = ROWS_PER_CORE // P                # 16
# stats for these (tile, j) run on the scalar engine (Copy/Square+accum)
SCALAR_STATS = {(1, 0)}
# applies for these tiles run on vector (idle once stats are done);
# scalar handles the rest
V_APPLY_TILES = {4, 5}

_nc_cache: dict = {}


def _build_nc(scale: float, affine: bool) -> bass.Bass:
    """Per-core program: rows (2048, 1024) fp16 -> LayerNorm -> fp16."""
    nc = bacc.Bacc("TRN2", target_bir_lowering=False, debug=False,
                   num_devices=N_CORES)
    x = nc.dram_tensor("x", [ROWS_PER_CORE, D], mybir.dt.float16,
                       kind="ExternalInput")
    out = nc.dram_tensor("out", [ROWS_PER_CORE, D], mybir.dt.float16,
                         kind="ExternalOutput")
    if affine:
        gamma = nc.dram_tensor("gamma", [P, D], mybir.dt.float32,
                               kind="ExternalInput")
        beta = nc.dram_tensor("beta", [P, D], mybir.dt.float32,
                              kind="ExternalInput")

    # tile i covers rows [128*off_i, 128*(off_i+njs)); within a tile,
    # partition p holds njs consecutive DRAM rows (contiguous DMA lines)

    FMAX = nc.vector.BN_STATS_FMAX          # 512
    n_sub = D // FMAX                       # 2
    SDIM = nc.vector.BN_STATS_DIM           # 6
    ADIM = nc.vector.BN_AGGR_DIM            # 2

    offs = [0]
    for njs in TILE_JS:
        offs.append(offs[-1] + njs)
    assert offs[-1] == N_J

    with TileContext(nc) as tc:
        with (
            tc.tile_pool(name="work", bufs=1) as work,
            tc.tile_pool(name="small", bufs=1) as small,
            tc.tile_pool(name="singles", bufs=1) as singles,
        ):
            # all input loads first (no deps; FIFO order = tile order)
            xts = []
            x_views = []
            out_views = []
            for i, njs in enumerate(TILE_JS):
                xv = x[P * offs[i]:P * offs[i + 1], :].rearrange(
                    "(p j) d -> p j d", j=njs)
                ov = out[P * offs[i]:P * offs[i + 1], :].rearrange(
                    "(p j) d -> p j d", j=njs)
                x_views.append(xv)
                out_views.append(ov)
                xt = work.tile([P, njs, D], mybir.dt.float16,
                               tag=f"xt{i}")
                nc.sync.dma_start(out=xt, in_=xv)
                xts.append(xt)

            eps_t = singles.tile([P, 1], mybir.dt.float32)
            nc.vector.memset(eps_t, EPS)
            sq_sink = singles.tile([P, D], mybir.dt.float16)
            # dummy Sqrt so the (single) ACT table set that covers
            # Sqrt+Square+Identity is loaded once, up front
            warm = singles.tile([P, 1], mybir.dt.float32)
            nc.scalar.activation(out=warm, in_=eps_t,
                                 func=mybir.ActivationFunctionType.Sqrt)
            if affine:
                gamma_t = singles.tile([P, D], mybir.dt.float32)
                beta_t = singles.tile([P, D], mybir.dt.float32)
                nc.sync.dma_start(out=gamma_t, in_=gamma[:, :])
                nc.sync.dma_start(out=beta_t, in_=beta[:, :])

            # per-tile state for the software pipeline
            state = []
            for i, njs in enumerate(TILE_JS):
                mv = small.tile([P, njs, ADIM], mybir.dt.float32,
                                tag=f"mv{i}")
                std = small.tile([P, njs], mybir.dt.float32,
                                 tag=f"std{i}")
                rstd = small.tile([P, njs], mybir.dt.float32,
                                  tag=f"rstd{i}")
                nmean = small.tile([P, njs], mybir.dt.float32,
                                   tag=f"nmean{i}")
                nmb = small.tile([P, njs], mybir.dt.float32,
                                 tag=f"nmb{i}")
                yt = work.tile([P, njs, D], mybir.dt.float16,
                               tag=f"yt{i}")
                state.append((mv, std, rstd, nmean, nmb, yt))

            def emit_stats(i):
                njs = TILE_JS[i]
                xt = xts[i]
                mv, std, rstd, nmean, nmb, _ = state[i]
                if scale != 1.0:
                    nc.scalar.mul(out=xt, in_=xt, mul=scale)
                stats = small.tile([P, njs, n_sub, SDIM],
                                   mybir.dt.float32, tag=f"stats{i}")
                acc = small.tile([P, njs, 2], mybir.dt.float32,
                                 tag=f"acc{i}")
                for j in range(njs):
                    if (i, j) in SCALAR_STATS:
                        nc.scalar.activation(
                            out=sq_sink, in_=xt[:, j, :],
                            func=mybir.ActivationFunctionType.Copy,
                            accum_out=acc[:, j, 0:1])
                        nc.scalar.activation(
                            out=sq_sink, in_=xt[:, j, :],
                            func=mybir.ActivationFunctionType.Square,
                            accum_out=acc[:, j, 1:2])
                        # mean = sums/D; var = ssq/D - mean^2  (vector)
                        nc.vector.tensor_scalar_mul(
                            out=mv[:, j, 0:1], in0=acc[:, j, 0:1],
                            scalar1=1.0 / D)
                        nc.vector.tensor_tensor(
                            out=acc[:, j, 0:1], in0=mv[:, j, 0:1],
                            in1=mv[:, j, 0:1], op=mybir.AluOpType.mult)
                        nc.vector.tensor_scalar_mul(
                            out=mv[:, j, 1:2], in0=acc[:, j, 1:2],
                            scalar1=1.0 / D)
                        nc.vector.tensor_tensor(
                            out=mv[:, j, 1:2], in0=mv[:, j, 1:2],
                            in1=acc[:, j, 0:1],
                            op=mybir.AluOpType.subtract)
                    else:
                        for k in range(n_sub):
                            nc.vector.bn_stats(
                                out=stats[:, j, k, :],
                                in_=xt[:, j, k * FMAX:(k + 1) * FMAX],
                            )
                        nc.vector.bn_aggr(out=mv[:, j, :],
                                          in_=stats[:, j, :, :])
                # nmean right away (only needs mv)
                nc.vector.tensor_scalar_mul(out=nmean, in0=mv[:, :, 0],
                                            scalar1=-1.0)
                nc.scalar.activation(
                    out=std, in_=mv[:, :, 1],
                    func=mybir.ActivationFunctionType.Sqrt,
                    bias=eps_t[:, 0:1], scale=1.0,
                )
                nc.vector.reciprocal(out=rstd, in_=std)
                nc.vector.tensor_tensor(out=nmb, in0=nmean, in1=rstd,
                                        op=mybir.AluOpType.mult)

            def emit_apply(i):
                njs = TILE_JS[i]
                xt = xts[i]
                mv, std, rstd, nmean, nmb, yt = state[i]
                store_every = 1 if njs <= 2 else 2
                for j0 in range(0, njs, store_every):
                    j1 = min(j0 + store_every, njs)
                    for j in range(j0, j1):
                        if i in V_APPLY_TILES:
                            nc.vector.tensor_scalar(
                                out=yt[:, j, :], in0=xt[:, j, :],
                                scalar1=mv[:, j, 0:1],
                                scalar2=rstd[:, j:j + 1],
                                op0=mybir.AluOpType.subtract,
                                op1=mybir.AluOpType.mult,
                            )
                        else:
                            nc.scalar.activation(
                                out=yt[:, j, :], in_=xt[:, j, :],
                                func=mybir.ActivationFunctionType.Identity,
                                bias=nmb[:, j:j + 1],
                                scale=rstd[:, j:j + 1],
                            )
                        if affine:
                            nc.vector.tensor_mul(out=yt[:, j, :],
                                                 in0=yt[:, j, :],
                                                 in1=gamma_t)
                            nc.vector.tensor_add(out=yt[:, j, :],
                                                 in0=yt[:, j, :],
                                                 in1=beta_t)
                    nc.sync.dma_start(out=out_views[i][:, j0:j1, :],
                                      in_=yt[:, j0:j1, :])

            # software pipeline: stats(i) ... then apply(i-1)
            emit_stats(0)
            for i in range(1, len(TILE_JS)):
                emit_stats(i)
                emit_apply(i - 1)
            emit_apply(len(TILE_JS) - 1)
    nc.compile()
    return nc


def _get_nc(scale: float, affine: bool) -> bass.Bass:
    key = (round(scale, 12), affine)
    if key not in _nc_cache:
        _nc_cache[key] = _build_nc(scale, affine)
    return _nc_cache[key]


def _preprocess(x, rotation_matrix, frequency_kernel):
    """Fold the frequency filter + rotation into (y, scale) on the host."""
    b, s, d = x.shape
    K = np.asarray(frequency_kernel, np.float64)[:s]
    h = np.fft.ifft(K).real
    y = x
    scale = float(h[0])
    if np.max(np.abs(h[1:])) > 1e-9 * max(1.0, np.max(np.abs(h))):
        xq = x.reshape(b, s, d // ROT, ROT)
        y = np.fft.ifft(np.fft.fft(xq, axis=1) * K.reshape(1, s, 1, 1),
                        axis=1).real.astype(np.float32).reshape(b, s, d)
        scale = 1.0
    R = np.asarray(rotation_matrix, np.float32)
    if not np.allclose(R, np.eye(ROT, dtype=np.float32), atol=1e-9):
        y = np.einsum("bstq,oq->bsto", y.reshape(b, s, d // ROT, ROT),
                      R).reshape(b, s, d).astype(np.float32)
    return np.ascontiguousarray(y, np.float32), scale


def run(x, rotation_matrix, frequency_kernel, ln_gamma, ln_beta,
        trace: bool = False, tmpdir: str | None = None):
    x = np.ascontiguousarray(np.asarray(x, np.float32))
    assert x.shape == (B, S, D), x.shape
    y, scale = _preprocess(x, rotation_matrix, frequency_kernel)
    if abs(scale - 1.0) < 1e-12:
        scale = 1.0
    g = np.asarray(ln_gamma, np.float32)
    bt = np.asarray(ln_beta, np.float32)
    affine = not (np.all(g == 1.0) and np.all(bt == 0.0))

    nc = _get_nc(scale, affine)
    y16 = y.astype(np.float16)
    # j-major shards: core c gets rows [c*2048, (c+1)*2048); within a
    # core, row r maps to (j = r // P, p = r % P)
    shards = y16.reshape(N_CORES, ROWS_PER_CORE, D)
    in_maps = []
    for c in range(N_CORES):
        m = {"x": shards[c]}
        if affine:
            m["gamma"] = np.ascontiguousarray(
                np.broadcast_to(g, (P, D)), np.float32)
            m["beta"] = np.ascontiguousarray(
                np.broadcast_to(bt, (P, D)), np.float32)
        in_maps.append(m)
    res = run_bass_kernel_spmd(nc, in_maps, list(range(N_CORES)),
                               trace=trace, tmpdir=tmpdir)
    out = np.stack([res.results[c]["out"] for c in range(N_CORES)])
    return out.reshape(B, S, D).astype(np.float32), res


def kernel(x, rotation_matrix, frequency_kernel, ln_gamma, ln_beta):
    out, _ = run(x, rotation_matrix, frequency_kernel, ln_gamma, ln_beta)
    return out
